# revision 1
# baseline (speedup 1.0000x reference)
"""Trainium2 Bass kernel for HPUSharedBiasGenerator.

out[t, b, j] = 0.0 if j < usage_grid[t, b] else -inf, where usage_grid is the
dense scatter of block_usages over (token_idx, block_idx). The 768 KiB of index
inputs are folded into a dense [qlen, blocks] grid on host (cheap); the 256 MiB
output is generated on 8 NeuronCores, sharded along the token dim.

Per core: 512 row-tiles of [128 rows, 128 j]. One fused DVE/Pool instruction per
tile writes the final bit pattern directly: (iota_j >= u) * -8388608 computed
into an int32-bitcast view gives 0x00000000 / 0xFF800000 == f32 0.0 / -inf.
Tiles are grouped K=16 per SBUF supertile and DMA'd out as single 1 MiB
transfers.
"""

import numpy as np

N_CORES = 8
QLEN = 8192
NBLK = 64
BS = 128
TOK_PER_CORE = QLEN // N_CORES          # 1024
ROWS_PER_CORE = TOK_PER_CORE * NBLK     # 65536
NTILES = ROWS_PER_CORE // 128           # 512
K = 16                                  # tiles per supertile (1 MiB DMA)
NSUPER = NTILES // K                    # 32

_CACHE = {}


def _build():
    import concourse.bacc as bacc
    import concourse.mybir as mybir
    from concourse.tile import TileContext

    nc = bacc.Bacc("TRN2", target_bir_lowering=False)
    u_in = nc.dram_tensor("u", [128, NTILES], mybir.dt.float32, kind="ExternalInput")
    out = nc.dram_tensor("out", [NTILES, 128, BS], mybir.dt.float32,
                         kind="ExternalOutput")
    with TileContext(nc) as tc:
        with (
            tc.tile_pool(name="const", bufs=1) as cpool,
            tc.tile_pool(name="work", bufs=4) as wpool,
        ):
            u_sb = cpool.tile([128, NTILES], mybir.dt.float32)
            nc.gpsimd.dma_start(u_sb[:], u_in[:])
            j_sb = cpool.tile([128, BS], mybir.dt.float32)
            nc.gpsimd.iota(j_sb[:], pattern=[[1, BS]], base=0, channel_multiplier=0,
                           allow_small_or_imprecise_dtypes=True)
            for s in range(NSUPER):
                st = wpool.tile([128, K * BS], mybir.dt.float32, tag="super")
                for k in range(K):
                    t = s * K + k
                    eng = nc.vector if k % 2 == 0 else nc.gpsimd
                    eng.tensor_scalar(
                        st[:, k * BS:(k + 1) * BS].bitcast(mybir.dt.int32),
                        j_sb[:], u_sb[:, t:t + 1], -8388608,
                        op0=mybir.AluOpType.is_ge, op1=mybir.AluOpType.mult)
                dst = out[s * K:(s + 1) * K].transpose([1, 0, 2])  # (p, k, j)
                src = st[:].rearrange("p (k j) -> p k j", k=K)
                nc.sync.dma_start(dst, src)
    nc.compile()
    return nc


def _get_nc():
    if "nc" not in _CACHE:
        _CACHE["nc"] = _build()
    return _CACHE["nc"]


def kernel(block_usages, hpu_shared_token_idx, hpu_shared_block_idx,
           block_size, target_qlen, target_shared_blocks, **kw):
    from concourse.bass_utils import run_bass_kernel_spmd

    usages = np.asarray(block_usages, dtype=np.float32)
    tok = np.asarray(hpu_shared_token_idx, dtype=np.int64)
    blk = np.asarray(hpu_shared_block_idx, dtype=np.int64)
    assert int(block_size) == BS and int(target_qlen) == QLEN
    assert int(target_shared_blocks) == NBLK

    # dense usage grid; rows never scattered keep usage 0 -> all -inf
    grid = np.zeros((QLEN, NBLK), dtype=np.float32)
    grid[tok, blk] = usages

    in_maps = []
    for c in range(N_CORES):
        shard = grid[c * TOK_PER_CORE:(c + 1) * TOK_PER_CORE].reshape(ROWS_PER_CORE)
        u_host = np.ascontiguousarray(shard.reshape(NTILES, 128).T)  # [128, NTILES]
        in_maps.append({"u": u_host})

    nc = _get_nc()
    res = run_bass_kernel_spmd(nc, in_maps, core_ids=list(range(N_CORES)))
    parts = [r["out"].reshape(TOK_PER_CORE, NBLK, BS) for r in res.results]
    return np.concatenate(parts, axis=0)


# revision 2
# speedup vs baseline: 769.6616x; 769.6616x over previous
"""Trainium2 Bass kernel for HPUSharedBiasGenerator.

out[t, b, j] = 0.0 if j < usage_grid[t, b] else -inf, where usage_grid is the
dense scatter of block_usages over (token_idx, block_idx). The 768 KiB of index
inputs are folded into a dense [qlen, blocks] grid on host (cheap); the 256 MiB
output is generated on 8 NeuronCores, sharded along the token dim (pure data
parallel, no cross-device comms).

Per-core layout: partition = token (128 tokens per group, 8 groups), free dim =
(block, j) so each partition's output is one token's contiguous 32 KiB row in
HBM. Two pipelined passes per chunk of 8 blocks:
  DVE : mask = (u <= j) via tensor_tensor with stride-0 broadcast APs
  ACT : value-cast mask * -8388608.0 into an int32 view -> bytes 0xFF800000,
        which is exactly f32 -inf (and 0.0 stays 0), no transcendental needed.
Then a 512 KiB HWDGE DMA per chunk with 4 KiB-contiguous runs per partition
(>= the bus-saturation descriptor size). TimelineSim: ~102 us/core, at the
aggregate DMA-bandwidth roofline for the 32 MiB/core output write.
"""

import numpy as np

N_CORES = 8
QLEN = 8192
NBLK = 64
BS = 128
TOK_PER_CORE = QLEN // N_CORES          # 1024
G = TOK_PER_CORE // 128                 # 8 token groups per core
CH = 8                                  # chunks per group
BC = NBLK // CH                         # 8 blocks per chunk
BUFS = 6

_CACHE = {}


def _build():
    import concourse.bacc as bacc
    import concourse.mybir as mybir
    from concourse.tile import TileContext

    nc = bacc.Bacc("TRN2", target_bir_lowering=False)
    u_in = nc.dram_tensor("u", [128, G * NBLK], mybir.dt.float32,
                          kind="ExternalInput")
    out = nc.dram_tensor("out", [G * 128, NBLK * BS], mybir.dt.float32,
                         kind="ExternalOutput")
    with TileContext(nc) as tc:
        with (
            tc.tile_pool(name="const", bufs=1) as cpool,
            tc.tile_pool(name="work", bufs=BUFS) as wpool,
        ):
            u_sb = cpool.tile([128, G * NBLK], mybir.dt.float32)
            nc.gpsimd.dma_start(u_sb[:], u_in[:])
            j_sb = cpool.tile([128, BS], mybir.dt.float32)
            nc.gpsimd.iota(j_sb[:], pattern=[[1, BS]], base=0,
                           channel_multiplier=0,
                           allow_small_or_imprecise_dtypes=True)
            for g in range(G):
                for h in range(CH):
                    mask = wpool.tile([128, BC * BS], mybir.dt.float32,
                                      tag="mask")
                    u_b = u_sb[:, g * NBLK + h * BC: g * NBLK + (h + 1) * BC]\
                        .unsqueeze(2).broadcast_to([128, BC, BS])
                    j_b = j_sb[:].unsqueeze(1).broadcast_to([128, BC, BS])
                    m3 = mask[:].rearrange("p (b j) -> p b j", b=BC)
                    nc.vector.tensor_tensor(m3, u_b, j_b,
                                            op=mybir.AluOpType.is_le)
                    ot = wpool.tile([128, BC * BS], mybir.dt.float32, tag="ot")
                    nc.scalar.mul(ot[:].bitcast(mybir.dt.int32), mask[:],
                                  -8388608.0)
                    nc.sync.dma_start(
                        out[g * 128:(g + 1) * 128,
                            h * BC * BS:(h + 1) * BC * BS],
                        ot[:])
    nc.compile()
    return nc


def _get_nc():
    if "nc" not in _CACHE:
        _CACHE["nc"] = _build()
    return _CACHE["nc"]


def kernel(block_usages, hpu_shared_token_idx, hpu_shared_block_idx,
           block_size, target_qlen, target_shared_blocks, **kw):
    from concourse.bass_utils import run_bass_kernel_spmd

    usages = np.asarray(block_usages, dtype=np.float32)
    tok = np.asarray(hpu_shared_token_idx, dtype=np.int64)
    blk = np.asarray(hpu_shared_block_idx, dtype=np.int64)
    assert int(block_size) == BS and int(target_qlen) == QLEN
    assert int(target_shared_blocks) == NBLK

    # dense usage grid; rows never scattered keep usage 0 -> all -inf
    grid = np.zeros((QLEN, NBLK), dtype=np.float32)
    grid[tok, blk] = usages

    in_maps = []
    for c in range(N_CORES):
        gc = grid[c * TOK_PER_CORE:(c + 1) * TOK_PER_CORE]  # [1024, 64]
        # u[p, g*64 + b] = gc[g*128 + p, b]
        u_host = np.ascontiguousarray(
            gc.reshape(G, 128, NBLK).transpose(1, 0, 2).reshape(128, G * NBLK))
        in_maps.append({"u": u_host})

    nc = _get_nc()
    res = run_bass_kernel_spmd(nc, in_maps, core_ids=list(range(N_CORES)))
    parts = [r["out"].reshape(TOK_PER_CORE, NBLK, BS) for r in res.results]
    return np.concatenate(parts, axis=0)
